# revision 21
# baseline (speedup 1.0000x reference)
"""Causal GQA self-attention (B=2,T=2048,C=4096, 32 q-heads, 8 kv-groups, hs=128)
sharded tensor-parallel across 8 TRN2 NeuronCores: one kv-group (4 q heads) per core.

v2: single dense PE stream with software-pipelined fill scheduling.
  seg1: qkv+rope for b=0 chunks (dense), v-transposes interleaved
  seg2: b=0 attention, PE bubbles filled with b=1 qkv matmul groups
  seg3: b=1 attention, filled with b=0 output-projection groups
  seg4: remaining projection, dense
Attention per (b,tcq,h): scores computed in 4-slice "quads" ([128,4,512] psum,
one exp ACTIVATE over all 2048 cols), diagonal quads use shrinking-N matmuls
(triangular), tril mask on gpsimd, denominator broadcast via ones-matrix lhs,
normalize = reciprocal + multiply on DVE (no PE broadcast matmul).
Host sums the 8 partial outputs in fp32.
"""
import math

import numpy as np
import ml_dtypes

import concourse.bass as bass
import concourse.mybir as mybir
import concourse.tile as tile
from concourse import bacc
from concourse.bass_utils import run_bass_kernel_spmd

BF16 = mybir.dt.bfloat16
F32 = mybir.dt.float32
AF = mybir.ActivationFunctionType

N_CORES = 8
B, T, C = 2, 2048, 4096
HS = 128
QPK = 4                  # q heads per kv group
GCOLS = (QPK + 2) * HS   # 768 qkv columns per group
TOK = B * T              # 4096
NCH = TOK // 512         # 8 token chunks of 512
SCALE = float(1.0 / np.sqrt(np.float32(HS)))

_NC_CACHE = None


def build_nc():
    nc = bacc.Bacc("TRN2", target_bir_lowering=False, debug=False,
                   num_devices=N_CORES)
    # host-packed layouts (see _prep_inputs)
    xt = nc.dram_tensor("xt", [128, NCH, 32, 512], BF16, kind="ExternalInput").ap()
    wq = nc.dram_tensor("wq", [128, 6, 32, 128], BF16, kind="ExternalInput").ap()
    wp = nc.dram_tensor("wp", [128, QPK, C], BF16, kind="ExternalInput").ap()
    # cos/sin slices per in-batch 512-chunk, bf16, sin pre-negated on first half
    cs = nc.dram_tensor("cs", [128, 4, 2, 512], BF16, kind="ExternalInput").ap()
    tri = nc.dram_tensor("tri", [128, 128], BF16, kind="ExternalInput").ap()
    ones = nc.dram_tensor("ones", [128, 128], BF16, kind="ExternalInput").ap()
    ident = nc.dram_tensor("ident", [128, 128], BF16, kind="ExternalInput").ap()
    out = nc.dram_tensor("out", [TOK, C], BF16, kind="ExternalOutput").ap()

    with tile.TileContext(nc) as tc:
        mid_cm = tc.tile_pool(name="mid", bufs=1)
        mid = mid_cm.__enter__()
        s2_cm = tc.tile_pool(name="s2", bufs=1)
        s2 = s2_cm.__enter__()
        pf_cm = tc.tile_pool(name="pf", bufs=2, space="PSUM")
        pf = pf_cm.__enter__()
        pscr_cm = tc.tile_pool(name="pscr", bufs=1, space="PSUM")
        pscr = pscr_cm.__enter__()
        po_cm = tc.tile_pool(name="po", bufs=1, space="PSUM")
        po = po_cm.__enter__()
        pd_cm = tc.tile_pool(name="pd", bufs=1, space="PSUM")
        pd = pd_cm.__enter__()
        s1_cm = tc.tile_pool(name="s1", bufs=1)
        s1 = s1_cm.__enter__()

        # ---- persistent sbuf ----
        qT = mid.tile([128, QPK, TOK], BF16)     # rope'd q, feature-major
        kT = mid.tile([128, TOK], BF16)          # rope'd k, feature-major
        ident_sb = mid.tile([128, 128], BF16)
        tri_sb = mid.tile([128, 128], BF16)
        ones_sb = mid.tile([128, 128], BF16)
        v_tok = s2.tile([128, 32, 128], BF16)    # v token-major per 128-slice
        yT = s2.tile([128, QPK, TOK], BF16)      # attention out, feature-major
        wq_sb = s1.tile([128, 6, 32, 128], BF16)

        nc.sync.dma_start(ident_sb[:], ident[:])
        nc.sync.dma_start(tri_sb[:], tri[:])
        nc.sync.dma_start(ones_sb[:], ones[:])
        # prewarm the exp table on ACT while DMAs stream
        warm = s2.tile([128, 128], BF16, tag="warm")
        nc.scalar.activation(warm[:], tri_sb[:], AF.Exp, scale=1.0)

        state = {"x": {}, "cs": {}, "vtmp": {}, "wp": None}

        def dma_chunk_piece(c, qi):
            xq = s1.tile([128, 4, 512], BF16, tag="x", bufs=15,
                         name=f"x{c}_{qi}")
            nc.sync.dma_start(xq[:], xt[:, c, qi * 4:(qi + 1) * 4, :])
            state["x"].setdefault(c, {})[qi] = xq

        def dma_chunk_cs(c):
            cst = s1.tile([128, 2, 512], BF16, tag="cs", bufs=1, name=f"cs{c}")
            nc.sync.dma_start(cst[:], cs[:, c % 4, :, :])
            state["cs"][c] = cst

        def dma_chunk(c):
            """Issue DMAs for chunk c's x (8 eighth tiles) + cos/sin."""
            for qi in range(8):
                dma_chunk_piece(c, qi)
            dma_chunk_cs(c)

        # startup: interleave wq m0/m1 pieces with x chunk-0 pieces so the
        # first k-loop can follow the DMA arrival curve
        for qr in range(4):
            nc.sync.dma_start(wq_sb[:, 0, qr * 8:(qr + 1) * 8, :],
                              wq[:, 0, qr * 8:(qr + 1) * 8, :])
            dma_chunk_piece(0, qr * 2)
            dma_chunk_piece(0, qr * 2 + 1)
            nc.sync.dma_start(wq_sb[:, 1, qr * 8:(qr + 1) * 8, :],
                              wq[:, 1, qr * 8:(qr + 1) * 8, :])
        dma_chunk_cs(0)
        for m in range(2, 6):
            for half in range(2):
                nc.sync.dma_start(wq_sb[:, m, half * 16:(half + 1) * 16, :],
                                  wq[:, m, half * 16:(half + 1) * 16, :])
        dma_chunk(1)

        def emit_s1_mm(c, m, kk, ps):
            xq = state["x"][c][kk // 4]
            nc.tensor.matmul(ps[:], wq_sb[:, m, kk, :], xq[:, kk % 4, :],
                             start=(kk == 0), stop=(kk == 31))

        def emit_s1_epilogue(c, m, ps):
            t0 = c * 512
            cst = state["cs"][c]
            if m == 5:                       # v: stage for transpose
                vtmp = s1.tile([128, 512], BF16, tag="vtmp", bufs=2,
                               name=f"vt{c}")
                nc.vector.tensor_copy(vtmp[:], ps[:])
                state["vtmp"][c] = vtmp
            else:                            # q heads 0-3 / k: rope
                t1 = s1.tile([128, 512], BF16, tag="t1", bufs=2, name="t1")
                nc.vector.tensor_mul(t1[:], ps[:], cst[:, 0, :])
                u = s1.tile([128, 512], BF16, tag="u", bufs=2, name="u")
                nc.vector.tensor_mul(u[0:64, :], ps[64:128, :],
                                     cst[0:64, 1, :])
                nc.vector.tensor_mul(u[64:128, :], ps[0:64, :],
                                     cst[64:128, 1, :])
                if m < 4:
                    dst = qT[:, m, t0:t0 + 512]
                else:
                    dst = kT[:, t0:t0 + 512]
                nc.vector.tensor_add(dst, t1[:], u[:])

        def emit_transpose_unit(c):
            """Transpose chunk c's v into token-major v_tok, scratching in the
            4-bank scores-quad psum tile (bitcast to bf16)."""
            vtmp = state["vtmp"][c]
            psq = pscr.tile([128, 4, 512], F32, tag="s", name=f"tp{c}")
            pb = psq.bitcast(BF16)           # [128, 4, 1024]
            for s in range(4):
                dst = pb[:, s, 0:128]
                nc.tensor.transpose(dst, vtmp[:, s * 128:(s + 1) * 128],
                                    ident_sb[:])
                nc.vector.tensor_copy(v_tok[:, c * 4 + s, :], dst)

        # fill queue: list of (kind, cycles, emit_fn)
        fillq = []

        def push_s1_chunk(c):
            if c + 1 < NCH:
                fillq.append(("dma", 0, lambda cc=c + 1: dma_chunk(cc)))
            for m in range(6):
                holder = {}
                for gi, (k0, k1) in enumerate(((0, 12), (12, 24), (24, 32))):
                    def fn(cc=c, mm=m, kk0=k0, kk1=k1, gi=gi, h=holder):
                        if gi == 0:
                            h["ps"] = pf.tile([128, 512], F32, tag="f",
                                              name=f"s1p{cc}_{mm}")
                        for kk in range(kk0, kk1):
                            emit_s1_mm(cc, mm, kk, h["ps"])
                        if kk1 == 32:
                            emit_s1_epilogue(cc, mm, h["ps"])
                    fillq.append(("s1", (k1 - k0) * 512, fn))
            fillq.append(("s1", 4 * 430, lambda cc=c: emit_transpose_unit(cc)))

        def push_proj(b, tcq):
            for ccg in range(8):
                for ti in range(b * 16 + tcq * 4, b * 16 + tcq * 4 + 4):
                    def fn(t=ti, cg=ccg):
                        wp_sb = state["wp"]
                        ps_p = pf.tile([128, 512], F32, tag="f",
                                       name=f"pj{t}_{cg}")
                        for h in range(QPK):
                            nc.tensor.matmul(
                                ps_p[:], yT[:, h, t * 128:(t + 1) * 128],
                                wp_sb[:, h, cg * 512:(cg + 1) * 512],
                                start=(h == 0), stop=(h == 3))
                        ob = state["s3"].tile([128, 512], BF16, tag="ob",
                                              bufs=6, name=f"ob{t}_{cg}")
                        if (t * 8 + cg) % 2 == 0:
                            nc.vector.tensor_copy(ob[:], ps_p[:])
                        else:
                            nc.scalar.activation(ob[:], ps_p[:], AF.Copy)
                        nc.sync.dma_start(
                            out[t * 128:(t + 1) * 128,
                                cg * 512:(cg + 1) * 512], ob[:])
                    fillq.append(("pj", 4 * 512, fn))

        fill_acct = {"spent": 0, "target": 0.0}

        def emit_fill(budget, kinds):
            # cumulative accounting: unit-granularity overshoot self-corrects
            fill_acct["target"] += budget
            while (fillq and fill_acct["spent"] < fill_acct["target"]
                   and fillq[0][0] in kinds):
                kind, cyc, fn = fillq.pop(0)
                fn()
                fill_acct["spent"] += cyc

        # ---------------- attention group ----------------
        def attention_group(b, tcq, h, fill_budget, kinds):
            t0g = b * T + tcq * 512
            n_s = (tcq + 1) * 4
            ps_o = po.tile([128, 512], F32, tag="o", name=f"o{b}{tcq}{h}")
            ps_d = pd.tile([128, 512], F32, tag="d", name=f"d{b}{tcq}{h}")
            for q in range(tcq + 1):
                diag = (q == tcq)
                ps_s = pscr.tile([128, 4, 512], F32, tag="s",
                                 name=f"s{b}{tcq}{h}{q}")
                offs = []
                for j in range(4):
                    si = q * 4 + j
                    off = 128 * j if diag else 0
                    offs.append(off)
                    s0g = b * T + si * 128
                    nc.tensor.matmul(
                        ps_s[:, j, off:512], kT[:, s0g:s0g + 128],
                        qT[:, h, t0g + off:t0g + 512],
                        start=True, stop=True)
                pt = s2.tile([128, 4, 512], BF16, tag="pt", bufs=2,
                             name=f"pt{b}{tcq}{h}{q}")
                nc.scalar.activation(pt[:], ps_s[:], AF.Exp, scale=SCALE)
                emit_fill(fill_budget, kinds)
                if diag:
                    for j in range(4):
                        o = 128 * j
                        nc.gpsimd.tensor_mul(pt[:, j, o:o + 128],
                                             pt[:, j, o:o + 128], tri_sb[:])
                for j in range(4):
                    si = q * 4 + j
                    off = offs[j]
                    nc.tensor.matmul(
                        ps_o[:, off:512], v_tok[:, b * 16 + si, :],
                        pt[:, j, off:512],
                        start=(si == 0), stop=(si == n_s - 1))
                    nc.tensor.matmul(
                        ps_d[:, off:512], ones_sb[:], pt[:, j, off:512],
                        start=(si == 0), stop=(si == n_s - 1))
            rden = s2.tile([128, 512], F32, tag="rd", bufs=1, name="rden")
            nc.vector.reciprocal_approx_fast(rden[:], ps_d[:])
            nc.vector.tensor_mul(yT[:, h, t0g:t0g + 512], ps_o[:], rden[:])

        # ================= emission =================
        # seg1: chunks 0-3 dense (b=0 qkv).  m-tiles processed in pairs with
        # split k-halves so late-arriving x quarters get 2x the DMA lead.
        for c in range(4):
            for ma, mb in ((0, 1), (2, 3), (4, 5)):
                if ma == 0 and c + 2 < 4:
                    dma_chunk(c + 2)
                psa = pf.tile([128, 512], F32, tag="f", name=f"c{c}m{ma}")
                psb = pf.tile([128, 512], F32, tag="f", name=f"c{c}m{mb}")
                for kk in range(16):
                    emit_s1_mm(c, ma, kk, psa)
                for kk in range(16):
                    emit_s1_mm(c, mb, kk, psb)
                for kk in range(16, 32):
                    emit_s1_mm(c, ma, kk, psa)
                emit_s1_epilogue(c, ma, psa)
                for kk in range(16, 32):
                    emit_s1_mm(c, mb, kk, psb)
                emit_s1_epilogue(c, mb, psb)
                if ma == 2 and c >= 1:
                    emit_transpose_unit(c - 1)
        emit_transpose_unit(3)

        # queue b=1 qkv as fill for seg2; prefetch chunk 4 now
        dma_chunk(4)
        for c in range(4, 8):
            push_s1_chunk(c)

        s1_cycles = sum(cyc for _, cyc, _ in fillq)
        # the qkv fills cover b=0's 40 quads plus (1,0)/(1,1)'s 12, so wproj
        # has the whole (1,0)+(1,1) stretch to stream in before proj fills
        n_cov = QPK * (sum(tcq + 1 for tcq in range(4)) + 1 + 2)   # 52
        budget0 = s1_cycles / n_cov

        # seg2: b=0 attention (then (1,0),(1,1)) + b=1 qkv fills
        for tcq in range(4):
            for h in range(QPK):
                attention_group(0, tcq, h, budget0, ("dma", "s1"))
            push_proj(0, tcq)
        for tcq in range(2):
            for h in range(QPK):
                attention_group(1, tcq, h, budget0, ("dma", "s1"))
            push_proj(1, tcq)

        # drain any qkv remainder before releasing the stage-1 pool
        while fillq and fillq[0][0] in ("dma", "s1"):
            _, _, fn = fillq.pop(0)
            fn()

        # close stage-1 pool, open proj pool; wproj arrives during b=1 attn
        # (leftover s1 fills drain first via the seg3 fill FIFO)
        s1_cm.__exit__(None, None, None)
        s3_cm = tc.tile_pool(name="s3", bufs=1)
        s3 = s3_cm.__enter__()
        state["s3"] = s3
        wp_sb = s3.tile([128, QPK, C], BF16)
        state["wp"] = wp_sb
        # column-block pieces so the first proj fills only need piece 0
        for cg in range(8):
            nc.sync.dma_start(wp_sb[:, :, cg * 512:(cg + 1) * 512],
                              wp[:, :, cg * 512:(cg + 1) * 512])

        n_rem = QPK * (3 + 4)                              # 28 quads left
        pj_cycles = sum(cyc for _, cyc, _ in fillq)
        budget1 = pj_cycles / n_rem
        fill_acct["spent"] = 0
        fill_acct["target"] = 0.0

        # seg3: b=1 attention (1,2),(1,3) + proj fills
        for tcq in range(2, 4):
            for h in range(QPK):
                attention_group(1, tcq, h, budget1, ("dma", "pj"))
            push_proj(1, tcq)

        # seg4: drain remaining proj
        while fillq:
            _, _, fn = fillq.pop(0)
            fn()

        for cm in (s3_cm, pd_cm, po_cm, pscr_cm, pf_cm, s2_cm, mid_cm):
            cm.__exit__(None, None, None)
    nc.compile()
    return nc


def _prep_inputs(x, cos, sin, Wqkv, Wproj):
    bf = ml_dtypes.bfloat16
    # x: [B,T,C] -> xT [C, TOK] -> [128p, chunk, 32ko, 512]
    xTn = x.reshape(TOK, C).T.astype(bf)                  # [C, TOK]
    xpack = np.ascontiguousarray(
        xTn.reshape(32, 128, NCH, 512).transpose(1, 2, 0, 3))
    # cos/sin: [T, 128] -> feature-major slices [128, 4tcq, 2, 512]
    cosT = cos.T.astype(np.float32)                       # [128, T]
    sinT = sin.T.astype(np.float32)
    sinb = np.concatenate([-sinT[0:64], sinT[64:128]], axis=0)
    cspack = np.empty((128, 4, 2, 512), dtype=np.float32)
    for tc in range(4):
        cspack[:, tc, 0, :] = cosT[:, tc * 512:(tc + 1) * 512]
        cspack[:, tc, 1, :] = sinb[:, tc * 512:(tc + 1) * 512]
    cspack = cspack.astype(bf)
    p = np.arange(128)[:, None]
    f = np.arange(128)[None, :]
    tri = (p <= f).astype(bf)                             # tril mask (kv<=q)
    ones = np.ones([128, 128], dtype=bf)
    ident = np.eye(128, dtype=np.float32).astype(bf)
    in_maps = []
    for g in range(N_CORES):
        Wg = np.ascontiguousarray(Wqkv[:, g * GCOLS:(g + 1) * GCOLS])
        # [C, 768] -> [128p, 6m, 32ko, 128]
        wqp = np.ascontiguousarray(
            Wg.reshape(32, 128, 6, 128).transpose(1, 2, 0, 3).astype(bf))
        Wpg = Wproj[g * 512:(g + 1) * 512, :]             # [512, C]
        wpp = np.ascontiguousarray(
            Wpg.reshape(QPK, 128, C).transpose(1, 0, 2).astype(bf))
        in_maps.append({
            "xt": xpack, "wq": wqp, "wp": wpp, "cs": cspack,
            "tri": tri, "ones": ones, "ident": ident,
        })
    return in_maps


def kernel(x, cos, sin, Wqkv, Wproj, _trace=False):
    global _NC_CACHE
    x = np.asarray(x, dtype=np.float32)
    cos = np.asarray(cos, dtype=np.float32)
    sin = np.asarray(sin, dtype=np.float32)
    Wqkv = np.asarray(Wqkv, dtype=np.float32)
    Wproj = np.asarray(Wproj, dtype=np.float32)
    if _NC_CACHE is None:
        _NC_CACHE = build_nc()
    nc = _NC_CACHE
    in_maps = _prep_inputs(x, cos, sin, Wqkv, Wproj)
    res = run_bass_kernel_spmd(nc, in_maps, core_ids=list(range(N_CORES)),
                               trace=_trace)
    acc = np.zeros([TOK, C], dtype=np.float32)
    for r in res.results:
        acc += r["out"].astype(np.float32)
    if _trace:
        kernel._last_exec_ns = res.exec_time_ns
        kernel._last_trace = res.instructions_and_trace
    return acc.reshape(B, T, C)
